# revision 1
# baseline (speedup 1.0000x reference)
"""Trainium2 Bass kernel for single-head cross-attention with additive mask.

Computation (matches the reference):
    q = tgt @ wq + bq
    k = src @ wk (+ bk dropped: softmax cancels a per-row constant exactly)
    v = src @ wv (bv folded into the epilogue: out = attn@v + bv)
    s = (q k^T + mask) / sqrt(DQ)
    out = softmax(s) @ v + bv

Two SPMD launches on 8 cores (the host glue between them is pure layout
shuffling -- concat / transpose / block-diagonal placement, no math):

  L1 (projections): each core projects k,v for 1/8 of the global (B*S) src
      rows and q for its L2 shard of tgt rows.  wk|wv are stacked into one
      [128,128] stationary operand so K^T and V^T come out of a single
      matmul stream (rows 0-63 = k, 64-127 = v).  All inputs are fp16
      (host-cast); rhs tiles are N=1024 wide so the PE streams at full rate.

  L2 (attention): tgt rows sharded 8 ways; core c handles tgt rows
      [c*512,(c+1)*512) of every batch so its mask slice is read from HBM
      exactly once.  Scores are built transposed (src rows on PSUM
      partitions) so the PV matmul consumes softmax weights directly.
      Per (batch-pair, src-block):
        - QK: lhsT = [k_b0; k_b1] stacked on 128 partitions against the
          block-diagonal rhs [[q_b0,0],[0,q_b1]], so both batches of a pair
          contract in full-width 128-row passes (two 512-col matmuls --
          one matmul output must fit a single 2KB PSUM bank).
        - the DVE adds the resident fp16 mask into the scores through a
          stride-0 broadcast view (one [128,2,512] add covers both batch
          halves; the mask is stored once, not duplicated).
        - ACT applies exp(0.125*x) emitting fp16 attention weights.
        - PV accumulates fp32 in PSUM; V carries a trailing ones-column so
          row 64 of the accumulator is the softmax denominator.
      Steady state is 3-way balanced: PE ~1.1us, DVE add ~1.14us, ACT exp
      ~1.14us per (pair, src-block) iteration.
      Epilogue: the sums row (partition 64) is broadcast down to partitions
      0-63 with a tiny ones-matmul on the idle PE (reciprocal_approx_* only
      works at partition base 0, and DVE lanes cannot shift partitions),
      then reciprocal + multiply + bv bias, store.  The output leaves
      transposed [B, DQ, TS]; the host flips it.
"""
import numpy as np

B, S, D, DQ = 4, 4096, 1024, 64
NCORES = 8
TS = S // NCORES            # 512 tgt rows per core per batch (L2 shard)
SR = (B * S) // NCORES      # 2048 global src rows per core (L1 shard)
SB = S // 128               # 32 src blocks per batch
GK = B * SB                 # 128 global src blocks
DQ1 = DQ + 1                # v65 block width (ones col + v)
CORES = list(range(NCORES))
F32 = np.float32
FP16 = np.float16

_CACHE = {}


def _build_l1():
    import concourse.mybir as mybir
    import concourse.tile as tile
    from concourse import bacc

    f32 = mybir.dt.float32
    fp16 = mybir.dt.float16
    AF = mybir.ActivationFunctionType

    nc = bacc.Bacc("TRN2", target_bir_lowering=False, debug=False,
                   num_devices=NCORES)
    srcT = nc.dram_tensor("srcT", [D, SR], fp16, kind="ExternalInput")
    tgtT = nc.dram_tensor("tgtT", [D, SR], fp16, kind="ExternalInput")
    wkv = nc.dram_tensor("wkv", [D, 2 * DQ], fp16, kind="ExternalInput")
    wq = nc.dram_tensor("wq", [D, DQ], fp16, kind="ExternalInput")
    bq = nc.dram_tensor("bq", [DQ], f32, kind="ExternalInput")
    # kvt rows 0-63 = k^T, rows 64-127 = v^T (s = this core's 2048 src rows)
    kvt = nc.dram_tensor("kvt", [2 * DQ, SR], fp16, kind="ExternalOutput")
    # qt cols = (b, t) for this core's 4x512 tgt rows
    qt = nc.dram_tensor("qt", [DQ, SR], fp16, kind="ExternalOutput")

    with tile.TileContext(nc) as tc:
        with (
            tc.tile_pool(name="const", bufs=1) as constp,
            tc.tile_pool(name="big", bufs=1) as bigp,
            tc.tile_pool(name="stream", bufs=2) as streamp,
            tc.tile_pool(name="pp", bufs=1, space="PSUM") as pp,
        ):
            wkv_sb = constp.tile([128, 8 * 2 * DQ], fp16)
            nc.sync.dma_start(
                out=wkv_sb.rearrange("p (j m) -> p j m", m=2 * DQ),
                in_=wkv.rearrange("(j p) m -> p j m", p=128))
            wq_sb = constp.tile([128, 8 * DQ], fp16)
            nc.sync.dma_start(
                out=wq_sb.rearrange("p (j m) -> p j m", m=DQ),
                in_=wq.rearrange("(j p) m -> p j m", p=128))
            bq_sb = constp.tile([DQ, 1], f32)
            nc.sync.dma_start(out=bq_sb[:], in_=bq.rearrange("(p o) -> p o", o=1))

            kv_ps = pp.tile([128, SR], f32, tag="kv")
            q_ps = pp.tile([DQ, SR], f32, tag="q")
            for j in range(8):
                st = streamp.tile([128, SR], fp16, tag="xs", bufs=8,
                                  name=f"st{j}")
                tg = streamp.tile([128, SR], fp16, tag="xt", bufs=8,
                                  name=f"tg{j}")
                if j < 2:
                    # fine-grained first chunks spread over many DMA queues
                    # (per-queue bandwidth is only ~20GB/s) so the PE
                    # unblocks quickly instead of waiting on whole tiles
                    engs = [nc.sync, nc.scalar, nc.gpsimd]
                    for h in range(8):
                        engs[h % 3].dma_start(
                            out=st[:, h * 256:(h + 1) * 256],
                            in_=srcT[j * 128:(j + 1) * 128,
                                     h * 256:(h + 1) * 256])
                        engs[(h + 1) % 3].dma_start(
                            out=tg[:, h * 256:(h + 1) * 256],
                            in_=tgtT[j * 128:(j + 1) * 128,
                                     h * 256:(h + 1) * 256])
                else:
                    for h in range(2):
                        nc.sync.dma_start(
                            out=st[:, h * 1024:(h + 1) * 1024],
                            in_=srcT[j * 128:(j + 1) * 128,
                                     h * 1024:(h + 1) * 1024])
                        nc.gpsimd.dma_start(
                            out=tg[:, h * 1024:(h + 1) * 1024],
                            in_=tgtT[j * 128:(j + 1) * 128,
                                     h * 1024:(h + 1) * 1024])
                for h in range(4):
                    nc.tensor.matmul(
                        kv_ps[:, h * 512:(h + 1) * 512],
                        lhsT=wkv_sb[:, j * 128:(j + 1) * 128],
                        rhs=st[:, h * 512:(h + 1) * 512],
                        start=(j == 0), stop=(j == 7))
                    nc.tensor.matmul(
                        q_ps[:, h * 512:(h + 1) * 512],
                        lhsT=wq_sb[:, j * DQ:(j + 1) * DQ],
                        rhs=tg[:, h * 512:(h + 1) * 512],
                        start=(j == 0), stop=(j == 7))
            kv_sb = bigp.tile([128, SR], fp16)
            q_sb = bigp.tile([DQ, SR], fp16)
            for h in range(2):
                sl = slice(h * 1024, (h + 1) * 1024)
                nc.vector.tensor_copy(kv_sb[:, sl], kv_ps[:, sl])
                nc.scalar.activation(q_sb[:, sl], q_ps[:, sl], AF.Identity,
                                     bias=bq_sb[:])
                nc.sync.dma_start(out=kvt[:, sl], in_=kv_sb[:, sl])
                nc.gpsimd.dma_start(out=qt[:, sl], in_=q_sb[:, sl])
    nc.compile()
    return nc


def _build_l2():
    import concourse.mybir as mybir
    import concourse.tile as tile
    from concourse import bacc

    f32 = mybir.dt.float32
    fp16 = mybir.dt.float16
    AF = mybir.ActivationFunctionType

    nc = bacc.Bacc("TRN2", target_bir_lowering=False, debug=False,
                   num_devices=NCORES)
    # kt2 cols pair*S + s; rows 0-63 = d of batch 2*pair, 64-127 = 2*pair+1
    kt2d = nc.dram_tensor("kt2", [128, 2 * S], fp16, kind="ExternalInput")
    # v65 block kg: cols 0..63 = v[kg*128 + p, :], col 64 = ones
    v65d = nc.dram_tensor("v65", [128, GK * DQ1], fp16, kind="ExternalInput")
    # block-diagonal q: qbd[pair] = [[q_b0^T, 0], [0, q_b1^T]]  (128 x 1024)
    qbdd = nc.dram_tensor("qbd", [2, 128, 2 * TS], fp16, kind="ExternalInput")
    # mask slice pre-arranged by the host into the exact SBUF layout
    # [128 partitions, (sb, t)] so the load is fully linear (4KB+ lines)
    maskPd = nc.dram_tensor("maskP", [128, SB * TS], fp16,
                            kind="ExternalInput")
    bv = nc.dram_tensor("bv", [DQ], f32, kind="ExternalInput")
    # transposed fp16 output: host flips [B, DQ, TS] -> [B, TS, DQ] and
    # upcasts to fp32 (pure layout/dtype glue)
    out = nc.dram_tensor("out", [B, DQ, TS], fp16, kind="ExternalOutput")

    with tile.TileContext(nc) as tc:
        with (
            tc.tile_pool(name="const", bufs=1) as constp,
            tc.tile_pool(name="big", bufs=1) as bigp,
            tc.tile_pool(name="stream", bufs=2) as streamp,
            tc.tile_pool(name="pp", bufs=1, space="PSUM") as pp,
        ):
            bv_sb = constp.tile([DQ, 1], f32)
            nc.sync.dma_start(out=bv_sb[:], in_=bv.rearrange("(p o) -> p o", o=1))
            # ones row at partition 64 (same base as the pv sums row)
            ones_sb = constp.tile([DQ1, DQ], f32)
            nc.vector.memset(ones_sb[DQ:DQ1, :], 1.0)

            # Resident loads, ordered so iteration 0 unblocks ASAP and each
            # src-block's data lands ahead of its loop iteration.
            qbd_sb = bigp.tile([128, 2 * 2 * TS], fp16)
            kt2 = bigp.tile([128, 2 * S], fp16)
            v65 = bigp.tile([128, GK * DQ1], fp16)
            # mask resident once; the DVE add reads it through a stride-0
            # broadcast view to cover both batch halves of a score tile
            msb = bigp.tile([128, SB * TS], fp16)

            def load_kt2(eng, c0, c1):
                eng.dma_start(out=kt2[:, c0:c1], in_=kt2d[:, c0:c1])

            def load_v65(eng, k0, k1):
                eng.dma_start(out=v65[:, k0 * DQ1:k1 * DQ1],
                              in_=v65d[:, k0 * DQ1:k1 * DQ1])

            def load_mask(eng, s0, s1):
                eng.dma_start(out=msb[:, s0 * TS:s1 * TS],
                              in_=maskPd[:, s0 * TS:s1 * TS])

            # iteration-0 critical chunks first, small and spread across
            # many DMA queues (per-queue bandwidth is only ~20GB/s)
            for i in range(4):
                eng = [nc.sync, nc.scalar, nc.gpsimd, nc.sync][i]
                eng.dma_start(
                    out=qbd_sb[:, i * 512:(i + 1) * 512],
                    in_=qbdd[i // 2, :, (i % 2) * 512:(i % 2 + 1) * 512])
            load_kt2(nc.scalar, 0, 256)      # pair0: sg 0-1
            load_mask(nc.gpsimd, 0, 1)
            load_v65(nc.sync, 0, 2)          # b0: kg 0-1
            load_v65(nc.scalar, 32, 34)      # b1: kg 32-33
            # near-term chunks
            load_mask(nc.gpsimd, 1, 4)
            load_kt2(nc.scalar, 256, 1024)
            load_v65(nc.sync, 2, 8)
            load_v65(nc.scalar, 34, 40)
            load_mask(nc.gpsimd, 4, 8)
            # bulk, in consumption order
            load_kt2(nc.sync, 1024, 4096)    # pair0 rest
            load_v65(nc.gpsimd, 8, 32)
            load_v65(nc.gpsimd, 40, 64)
            load_mask(nc.gpsimd, 8, 20)
            load_mask(nc.gpsimd, 20, 32)
            load_kt2(nc.sync, 4096, 8192)    # pair1
            load_v65(nc.sync, 64, 128)

            # main loop: batch-pair outer; scores stay transposed
            for pair in range(2):
                pv = [pp.tile([DQ1, TS], f32, tag=f"pv{h}",
                              name=f"pv{pair}_{h}") for h in range(2)]
                for sg in range(SB):
                    qkt = pp.tile([128, 2 * TS], f32, tag="qk", bufs=3,
                                  name=f"qkt{pair}_{sg}")
                    for half in range(2):
                        nc.tensor.matmul(
                            qkt[:, half * TS:(half + 1) * TS],
                            lhsT=kt2[:, pair * S + sg * 128:
                                     pair * S + sg * 128 + 128],
                            rhs=qbd_sb[:, pair * 1024 + half * TS:
                                       pair * 1024 + (half + 1) * TS],
                            start=True, stop=True)
                    es = streamp.tile([128, 2 * TS], fp16, tag="E", bufs=4,
                                      name=f"es{pair}_{sg}")
                    nc.vector.tensor_add(
                        es.rearrange("p (h t) -> p h t", h=2),
                        qkt.rearrange("p (h t) -> p h t", h=2),
                        msb[:, None, sg * TS:(sg + 1) * TS]
                        .broadcast_to([128, 2, TS]))
                    pt = streamp.tile([128, 2 * TS], fp16, tag="P", bufs=4,
                                      name=f"pt{pair}_{sg}")
                    nc.scalar.activation(pt[:], es[:], AF.Exp, scale=0.125)
                    for half in range(2):
                        kg = (pair * 2 + half) * SB + sg
                        nc.tensor.matmul(
                            pv[half][:],
                            lhsT=v65[:, kg * DQ1:(kg + 1) * DQ1],
                            rhs=pt[:, half * TS:(half + 1) * TS],
                            start=(sg == 0), stop=(sg == SB - 1))

                # epilogue: out^T = pv[0:64]/pv[64] + bv.  The sums row sits
                # on partition 64 but reciprocal_approx_* only works at
                # partition base 0 (custom-DVE uop), and DVE lanes can't
                # shift partitions -- so broadcast the sums down to
                # partitions 0-63 with a tiny ones-matmul on the idle PE,
                # then reciprocal + multiply lane-aligned with the values.
                for half in range(2):
                    b = pair * 2 + half
                    pvs = streamp.tile([DQ1, TS], f32, tag="pvs",
                                       name=f"pvs{b}")
                    nc.scalar.copy(pvs[:], pv[half][:])
                    rb = pp.tile([DQ, TS], f32, tag="pv0", bufs=1,
                                 name=f"rb{b}")
                    nc.tensor.matmul(rb[:], lhsT=ones_sb[DQ:DQ1, :],
                                     rhs=pvs[DQ:DQ1, :],
                                     start=True, stop=True)
                    sums_sb = streamp.tile([DQ, TS], f32, tag="sums",
                                           name=f"sums{b}")
                    nc.scalar.copy(sums_sb[:], rb[:])
                    recip = streamp.tile([DQ, TS], f32, tag="recip",
                                         name=f"recip{b}")
                    nc.vector.reciprocal_approx_fast(recip[:], sums_sb[:])
                    ot = streamp.tile([DQ, TS], f32, tag="ot", name=f"ot{b}")
                    nc.vector.tensor_mul(ot[:], pvs[0:DQ, :], recip[:])
                    of = streamp.tile([DQ, TS], fp16, tag="of",
                                      name=f"of{b}")
                    nc.scalar.activation(of[:], ot[:], AF.Identity,
                                         bias=bv_sb[:])
                    nc.gpsimd.dma_start(out=out[b], in_=of[:])
    nc.compile()
    return nc


def _get_l1():
    if "l1" not in _CACHE:
        _CACHE["l1"] = _build_l1()
    return _CACHE["l1"]


def _get_l2():
    if "l2" not in _CACHE:
        _CACHE["l2"] = _build_l2()
    return _CACHE["l2"]


def make_in_maps_l1(src, tgt, wk, wv, wq, bq):
    src_flat = np.asarray(src, dtype=F32).reshape(B * S, D)
    wkv = np.concatenate([np.asarray(wk, dtype=F32),
                          np.asarray(wv, dtype=F32)], axis=1).astype(FP16)
    wq16 = np.asarray(wq, dtype=F32).astype(FP16)
    bq = np.ascontiguousarray(bq, dtype=F32)
    tgt = np.asarray(tgt, dtype=F32)
    maps = []
    for c in CORES:
        # tgtT cols (b, t) for this core's L2 shard of tgt rows
        tslice = tgt[:, c * TS:(c + 1) * TS, :]         # [B, TS, D]
        tgtT = tslice.transpose(2, 0, 1).reshape(D, B * TS)
        maps.append({
            "srcT": np.ascontiguousarray(
                src_flat[c * SR:(c + 1) * SR, :].T.astype(FP16)),
            "tgtT": np.ascontiguousarray(tgtT.astype(FP16)),
            "wkv": wkv, "wq": wq16, "bq": bq,
        })
    return maps


def glue_l1_outputs(results):
    """Assemble L2's kt2 / v65 / per-core qbd from the 8 L1 outputs."""
    kvs = [np.asarray(results[c]["kvt"]) for c in CORES]
    kT_full = np.concatenate([kv[0:DQ] for kv in kvs], axis=1)    # [64, B*S]
    vT_full = np.concatenate([kv[DQ:2 * DQ] for kv in kvs], axis=1)
    # kt2: [128, 2S]; cols pair*S+s; rows 0-63 = batch 2p, 64-127 = 2p+1
    kt2 = np.empty((128, 2 * S), dtype=FP16)
    for pair in range(2):
        kt2[0:DQ, pair * S:(pair + 1) * S] = \
            kT_full[:, (2 * pair) * S:(2 * pair + 1) * S]
        kt2[DQ:128, pair * S:(pair + 1) * S] = \
            kT_full[:, (2 * pair + 1) * S:(2 * pair + 2) * S]
    v_full = vT_full.T                                            # [B*S, 64]
    v65 = np.empty((B * S, DQ1), dtype=FP16)
    v65[:, :DQ] = v_full
    v65[:, DQ] = np.asarray(1.0, dtype=FP16)
    v65 = np.ascontiguousarray(
        v65.reshape(GK, 128, DQ1).transpose(1, 0, 2).reshape(128, -1))
    # per-core block-diagonal q
    qbds = []
    for c in CORES:
        q = np.asarray(results[c]["qt"])                          # [64, B*TS]
        qbd = np.zeros((2, 128, 2 * TS), dtype=FP16)
        for bt in range(B):
            pair, h = bt // 2, bt % 2
            qbd[pair, h * DQ:(h + 1) * DQ, h * TS:(h + 1) * TS] = \
                q[:, bt * TS:(bt + 1) * TS]
        qbds.append(qbd)
    return np.ascontiguousarray(kt2), v65, qbds


def make_in_maps_l2(kt2, v65, qbds, mask, bv):
    mask = np.asarray(mask, dtype=F32)
    bv = np.ascontiguousarray(bv, dtype=F32)
    maps = []
    for c in CORES:
        mT = mask[c * TS:(c + 1) * TS, :].T.astype(FP16)   # [S, TS]
        mP = np.ascontiguousarray(
            mT.reshape(SB, 128, TS).transpose(1, 0, 2).reshape(128, SB * TS))
        maps.append({"kt2": kt2, "v65": v65, "qbd": qbds[c],
                     "maskP": mP, "bv": bv})
    return maps


def kernel(src, tgt, mask, wq, bq, wk, bk, wv, bv):
    from concourse.bass_utils import run_bass_kernel_spmd

    res1 = run_bass_kernel_spmd(
        _get_l1(), make_in_maps_l1(src, tgt, wk, wv, wq, bq), core_ids=CORES)
    kt2, v65, qbds = glue_l1_outputs(res1.results)
    res2 = run_bass_kernel_spmd(
        _get_l2(), make_in_maps_l2(kt2, v65, qbds, mask, bv), core_ids=CORES)
    out = np.empty((B, S, DQ), dtype=F32)
    for c in CORES:
        out[:, c * TS:(c + 1) * TS, :] = \
            np.asarray(res2.results[c]["out"]).transpose(0, 2, 1)
    return out

